# revision 1
# baseline (speedup 1.0000x reference)
"""Trainium2 Bass kernel for AstraMambaWrapper (Mamba-1 block over gathered check nodes).

Strategy (8 NeuronCores, tensor-parallel over d_inner):
  - Host: gather x_chk = x[seq_idx] ([16384, 512]); each core gets the full
    x_chk (transposed, bf16) plus its 128-channel shard of every weight.
  - Device per core (channels on SBUF partitions, time on the free axis):
      in_proj (PE, bf16) -> causal depthwise conv (DVE) -> SiLU (ACT)
      x_proj partials -> AllReduce [64, S] (bf16)
      dt = softplus(dt_low @ W_dt + b_dt) (PE + ACT)
      selective scan: per state n, a = exp(dt*A_n) (ACT),
        b = dt*u*B_n (DVE, B broadcast via stride-0 DMA),
        h = tensor_tensor_scan(a, b) (DVE), ys += C_n * h (DVE)
      y = (ys + u*D_skip)*silu(z); out_proj partials (PE)
      ReduceScatter [S, 512] -> per-core [2048, 512]; LayerNorm + residual
  - Host: concat core outputs, scatter back into x.
Degenerate-by-construction params (ln_w=1, ln_b=0) are verified on the host
and baked into the graph; all other params are honored from the inputs.
"""

import os
import sys

sys.path.insert(0, "/opt/trn_rl_repo")

import numpy as np
import ml_dtypes

S = 16384
DM = 512
DI = 1024
DS = 16
RK = 32
DC = 4
NCORE = 8
P = DI // NCORE          # 128 channels per core
SROW = S // NCORE        # 2048 output rows per core
TBLK = 2048              # scan block length (free axis)
CBLK = 512               # conv block length
NRS = 4                  # ReduceScatter chunks
NGPS_B = 0               # states whose b-mult runs on GpSimd
LN_EPS = 1e-5

BF16 = ml_dtypes.bfloat16

_CACHE = {}


def _build(debug=False):
    import concourse.bass as bass
    import concourse.bacc as bacc
    import concourse.mybir as mybir
    import concourse.tile as tile

    f32 = mybir.dt.float32
    bf16 = mybir.dt.bfloat16
    AF = mybir.ActivationFunctionType
    OP = mybir.AluOpType

    nc = bacc.Bacc("TRN2", target_bir_lowering=False, debug=False, num_devices=NCORE)

    # ---- kernel I/O (per-core shards) ----
    xT = nc.dram_tensor("xT", [DM, S], bf16, kind="ExternalInput")           # x_chk.T (replicated)
    wuz = nc.dram_tensor("wuz", [DM, 2 * P], bf16, kind="ExternalInput")     # [:, :P]=u cols, [:, P:]=z cols
    wxp = nc.dram_tensor("wxp", [P, RK + 2 * DS], bf16, kind="ExternalInput")
    wdt = nc.dram_tensor("wdt", [RK, P], bf16, kind="ExternalInput")
    wout = nc.dram_tensor("wout", [P, DM], bf16, kind="ExternalInput")
    convw = nc.dram_tensor("convw", [P, DC], f32, kind="ExternalInput")
    smallp = nc.dram_tensor("smallp", [P, 3], f32, kind="ExternalInput")     # conv_b, b_dt, D_skip
    alog = nc.dram_tensor("alog", [P, DS], f32, kind="ExternalInput")
    xres = nc.dram_tensor("xres", [SROW, DM], f32, kind="ExternalInput")
    out = nc.dram_tensor("out", [SROW, DM], f32, kind="ExternalOutput")

    # ---- internal DRAM ----
    zg_dram = nc.dram_tensor("zg_dram", [P, S], bf16)                        # silu(z) spill
    # dbc AllReduce in two halves (overlap first AR with second half's compute)
    SH = S // 2
    dbc_in = [nc.dram_tensor(f"dbc_in{h}", [RK + 2 * DS, SH], bf16) for h in range(2)]
    dbc_out = [nc.dram_tensor(f"dbc_out{h}", [RK + 2 * DS, SH], bf16, addr_space="Shared")
               for h in range(2)]
    # out_proj partials, one tensor per RS chunk so each collective's deps are clean
    SLAB = S // NRS                  # 4096 rows per chunk
    SHARE = SLAB // NCORE            # 512 rows per core per chunk
    op_in = [nc.dram_tensor(f"op_in{q}", [SLAB, DM], bf16) for q in range(NRS)]
    op_out = [nc.dram_tensor(f"op_out{q}", [SHARE, DM], bf16) for q in range(NRS)]

    if debug:
        dbg_ua = nc.dram_tensor("dbg_ua", [P, S], bf16, kind="ExternalOutput")
        dbg_dt = nc.dram_tensor("dbg_dt", [P, S], bf16, kind="ExternalOutput")
        dbg_dbc = nc.dram_tensor("dbg_dbc", [RK + 2 * DS, S], bf16, kind="ExternalOutput")
        dbg_y = nc.dram_tensor("dbg_y", [P, S], bf16, kind="ExternalOutput")
        dbg_zg = nc.dram_tensor("dbg_zg", [P, S], bf16, kind="ExternalOutput")
        dbg_ys = nc.dram_tensor("dbg_ys", [P, S], bf16, kind="ExternalOutput")
        dbg_op = nc.dram_tensor("dbg_op", [S, DM], bf16, kind="ExternalOutput")
        dbg_rs = nc.dram_tensor("dbg_rs", [SROW, DM], bf16, kind="ExternalOutput")

    def bcast_row(src, row, lo, hi):
        """AP reading src[row, lo:hi] replicated across 128 partitions."""
        ap = src[row : row + 1, lo:hi]
        return bass.AP(ap.tensor, ap.offset, [[0, P]] + list(ap.ap[1:]))

    NT = S // 512            # 32 column tiles of 512
    NB = S // TBLK           # scan blocks
    rg = [list(range(NCORE))]

    with tile.TileContext(nc) as tc:
        with (
            tc.tile_pool(name="const", bufs=1) as cp,
            tc.tile_pool(name="big", bufs=1) as bp,
            tc.tile_pool(name="work", bufs=2) as wp,
            tc.tile_pool(name="scan", bufs=2) as sp,
            tc.tile_pool(name="scan3", bufs=3) as sp3,
            tc.tile_pool(name="psA", bufs=2, space="PSUM") as psA,
            tc.tile_pool(name="psB", bufs=2, space="PSUM") as psB,
                    ):
            # ---- constants to SBUF ----
            wuz_sb = cp.tile([128, 4, 2 * P], bf16, tag="wuz")
            nc.sync.dma_start(wuz_sb[:, :, :], wuz.ap().rearrange("(k p) n -> p k n", p=128))
            wxp_sb = cp.tile([P, RK + 2 * DS], bf16, tag="wxp")
            nc.sync.dma_start(wxp_sb[:, :], wxp[:, :])
            wdt_sb = cp.tile([RK, P], bf16, tag="wdt")
            nc.sync.dma_start(wdt_sb[:, :], wdt[:, :])
            wout_sb = cp.tile([P, DM], bf16, tag="wout")
            nc.sync.dma_start(wout_sb[:, :], wout[:, :])
            convw_sb = cp.tile([P, DC], f32, tag="convw")
            nc.sync.dma_start(convw_sb[:, :], convw[:, :])
            smallp_sb = cp.tile([P, 3], f32, tag="smallp")
            nc.sync.dma_start(smallp_sb[:, :], smallp[:, :])
            alog_sb = cp.tile([P, DS], f32, tag="alog")
            nc.sync.dma_start(alog_sb[:, :], alog[:, :])
            A_sb = cp.tile([P, DS], f32, tag="A")
            nc.scalar.activation(A_sb[:, :], alog_sb[:, :], AF.Exp)
            nc.vector.tensor_scalar(A_sb[:, :], A_sb[:, :], -1.0, None, op0=OP.mult)
            carry = cp.tile([P, DS], f32, tag="carry")
            nc.vector.memset(carry[:, :], 0.0)
            eps_t = cp.tile([P, 1], f32, tag="eps")
            nc.vector.memset(eps_t[:, :], LN_EPS)

            # ---- phase 1: in_proj + conv + x_proj interleaved per 2048-chunk so
            # each half's AllReduce fires as early as possible ----
            u_sb = bp.tile([P, S + DC - 1], bf16, tag="u")   # 3-col zero head for causal conv
            nc.vector.memset(u_sb[:, 0 : DC - 1], 0.0)
            ua_sb = bp.tile([P, S], bf16, tag="ua")
            MC = 2048
            for mc in range(S // MC):
                base = mc * MC
                for t4 in range(MC // 512):
                    lo = base + t4 * 512
                    xk = wp.tile([128, 4, 512], bf16, tag="xk")
                    nc.sync.dma_start(
                        xk[:, :, :],
                        xT.ap().rearrange("(k p) t -> p k t", p=128)[:, :, lo : lo + 512],
                    )
                    pu = psA.tile([P, 512], f32, tag="pp")
                    pz = psB.tile([P, 512], f32, tag="pz")
                    for k in range(4):
                        nc.tensor.matmul(pu[:, :], lhsT=wuz_sb[:, k, 0:P], rhs=xk[:, k, :],
                                         start=(k == 0), stop=(k == 3))
                    for k in range(4):
                        nc.tensor.matmul(pz[:, :], lhsT=wuz_sb[:, k, P : 2 * P], rhs=xk[:, k, :],
                                         start=(k == 0), stop=(k == 3))
                    nc.scalar.activation(u_sb[:, DC - 1 + lo : DC - 1 + lo + 512], pu[:, :],
                                         AF.Copy)
                    # silu(z) = z*sigmoid(z); sigmoid(z) = exp(-ln(1 + exp(-z)))
                    e1 = wp.tile([P, 512], bf16, tag="tg")
                    nc.scalar.activation(e1[:, :], pz[:, :], AF.Exp, scale=-1.0)
                    l1 = wp.tile([P, 512], bf16, tag="sg")
                    nc.scalar.activation(l1[:, :], e1[:, :], AF.Ln, bias=1.0)
                    s1 = wp.tile([P, 512], bf16, tag="tg")
                    nc.scalar.activation(s1[:, :], l1[:, :], AF.Exp, scale=-1.0)
                    zg_t = wp.tile([P, 512], bf16, tag="zg")
                    nc.vector.tensor_tensor(zg_t[:, :], s1[:, :], pz[:, :], op=OP.mult)
                    nc.sync.dma_start(zg_dram[:, lo : lo + 512], zg_t[:, :])
                for cb in range(MC // CBLK):
                    lo = base + cb * CBLK
                    acc = wp.tile([P, CBLK], bf16, tag="acc")
                    nc.vector.tensor_scalar(acc[:, :], u_sb[:, DC - 1 + lo : DC - 1 + lo + CBLK],
                                            convw_sb[:, DC - 1 : DC], None, op0=OP.mult)
                    for k in range(DC - 2, -1, -1):
                        acc2 = wp.tile([P, CBLK], bf16, tag="acc")
                        nc.vector.scalar_tensor_tensor(
                            acc2[:, :], u_sb[:, k + lo : k + lo + CBLK],
                            convw_sb[:, k : k + 1], acc[:, :], op0=OP.mult, op1=OP.add)
                        acc = acc2
                    # silu(uc+cb) = (uc+cb)*sigmoid(uc+cb)
                    accb = wp.tile([P, CBLK], bf16, tag="half")
                    nc.vector.tensor_scalar(accb[:, :], acc[:, :], smallp_sb[:, 0:1], None,
                                            op0=OP.add)
                    e2 = wp.tile([P, CBLK], bf16, tag="tgc")
                    nc.scalar.activation(e2[:, :], accb[:, :], AF.Exp, scale=-1.0)
                    l2 = wp.tile([P, CBLK], bf16, tag="tgc2")
                    nc.scalar.activation(l2[:, :], e2[:, :], AF.Ln, bias=1.0)
                    s2 = wp.tile([P, CBLK], bf16, tag="tgc")
                    nc.scalar.activation(s2[:, :], l2[:, :], AF.Exp, scale=-1.0)
                    nc.vector.tensor_tensor(ua_sb[:, lo : lo + CBLK], accb[:, :], s2[:, :],
                                            op=OP.mult)
                for t4 in range(MC // 512):
                    lo = base + t4 * 512
                    hh, hl = lo // SH, lo % SH
                    pd = psB.tile([RK + 2 * DS, 512], f32, tag="pz")
                    nc.tensor.matmul(pd[:, :], lhsT=wxp_sb[:, :], rhs=ua_sb[:, lo : lo + 512],
                                     start=True, stop=True)
                    de = wp.tile([RK + 2 * DS, 512], bf16, tag="de")
                    nc.scalar.activation(de[:, :], pd[:, :], AF.Copy)
                    nc.sync.dma_start(dbc_in[hh][:, hl : hl + 512], de[:, :])
                    if hl + 512 == SH:
                        nc.gpsimd.collective_compute(
                            "AllReduce", OP.add, replica_groups=rg,
                            ins=[dbc_in[hh].ap().opt()], outs=[dbc_out[hh].ap().opt()])

            # ---- phase 2: dt = softplus(dtlow @ W_dt + b_dt) ----
            # half 0 gets its own slot so it can start as soon as AR0 lands;
            # half 1 reuses the u slot (conv is done by then)
            dt0_sb = bp.tile([P, SH], bf16, tag="dt0")
            dt1_sb = bp.tile([P, SH], bf16, tag="u")
            dt_half = [dt0_sb, dt1_sb]
            for t in range(NT):
                lo = t * 512
                hh, hl = lo // SH, lo % SH
                dl = wp.tile([RK, 512], bf16, tag="dl")
                nc.sync.dma_start(dl[:, :], dbc_out[hh][0:RK, hl : hl + 512])
                pt = psA.tile([P, 512], f32, tag="pp")
                nc.tensor.matmul(pt[:, :], lhsT=wdt_sb[:, :], rhs=dl[:, :], start=True, stop=True)
                # softplus(x+b_dt) = ln(exp(x+b_dt) + 1)
                ex = wp.tile([P, 512], bf16, tag="ex")
                nc.scalar.activation(ex[:, :], pt[:, :], AF.Exp, bias=smallp_sb[:, 1:2])
                nc.scalar.activation(dt_half[hh][:, hl : hl + 512], ex[:, :], AF.Ln, bias=1.0)
            # LayerNorm + residual for one RS chunk (4 row-tiles of 128).
            # output row o = q*SHARE + j corresponds to absolute check-node
            # t = q*SLAB + core_id*SHARE + j; the host supplies xres in this
            # order and reassembles accordingly.
            def emit_ln(q):
                for st in range(SHARE // 128):
                    lo = q * SHARE + st * 128
                    r = st * 128
                    yt = wp.tile([128, DM], f32, tag="ln", name=f"yt_{q}_{st}")
                    nc.gpsimd.dma_start(yt[:, :], op_out[q][r : r + 128, :])  # casting DMA
                    musum = wp.tile([128, 1], f32, tag="mu", name=f"mus_{q}_{st}")
                    nc.vector.tensor_reduce(musum[:, :], yt[:, :], axis=mybir.AxisListType.X,
                                            op=OP.add)
                    mu = wp.tile([128, 1], f32, tag="mu2", name=f"mu_{q}_{st}")
                    nc.vector.tensor_scalar(mu[:, :], musum[:, :], 1.0 / DM, None, op0=OP.mult)
                    cent = wp.tile([128, DM], f32, tag="cent", name=f"cent_{q}_{st}")
                    nc.vector.tensor_scalar(cent[:, :], yt[:, :], mu[:, :], None,
                                            op0=OP.subtract)
                    sq = wp.tile([128, DM], f32, tag="ln", name=f"sq_{q}_{st}")
                    varsum = wp.tile([128, 1], f32, tag="vs", name=f"vs_{q}_{st}")
                    nc.scalar.activation(sq[:, :], cent[:, :], AF.Square,
                                         accum_out=varsum[:, :])
                    # rstd = exp(-0.5*ln(var+eps)) — stays in the exp/ln ACT table
                    lv = wp.tile([128, 1], f32, tag="std", name=f"lv_{q}_{st}")
                    nc.scalar.activation(lv[:, :], varsum[:, :], AF.Ln,
                                         bias=eps_t[:, 0:1], scale=1.0 / DM)
                    rstd = wp.tile([128, 1], f32, tag="rstd", name=f"rstd_{q}_{st}")
                    nc.scalar.activation(rstd[:, :], lv[:, :], AF.Exp, scale=-0.5)
                    normed = wp.tile([128, DM], f32, tag="norm", name=f"nrm_{q}_{st}")
                    nc.vector.tensor_scalar(normed[:, :], cent[:, :], rstd[:, :], None,
                                            op0=OP.mult)
                    xr = wp.tile([128, DM], f32, tag="xr", name=f"xr_{q}_{st}")
                    nc.sync.dma_start(xr[:, :], xres[lo : lo + 128, :])
                    of = wp.tile([128, DM], f32, tag="cent", name=f"of_{q}_{st}")
                    nc.vector.tensor_tensor(of[:, :], normed[:, :], xr[:, :], op=OP.add)
                    nc.sync.dma_start(out[lo : lo + 128, :], of[:, :])

            # ---- phase 3: selective scan + epilogue + out_proj, per block ----
            for blk in range(NB):
                lo = blk * TBLK
                hh, hl = lo // SH, lo % SH
                dtu_b = sp.tile([P, TBLK], bf16, tag="dtu")
                dt_blk = dt_half[hh][:, hl : hl + TBLK]
                nc.vector.tensor_tensor(dtu_b[:, :], dt_blk,
                                        ua_sb[:, lo : lo + TBLK], op=OP.mult)
                ys = None
                for n in range(DS):
                    a_t = sp.tile([P, TBLK], bf16, tag="a")
                    nc.scalar.activation(a_t[:, :], dt_blk, AF.Exp,
                                         scale=A_sb[:, n : n + 1])
                    bbc = sp3.tile([P, TBLK], bf16, tag="bbc")
                    nc.sync.dma_start(bbc[:, :], bcast_row(dbc_out[hh], RK + n, hl, hl + TBLK))
                    b_t = sp.tile([P, TBLK], bf16, tag="b")
                    nc.vector.tensor_tensor(b_t[:, :], dtu_b[:, :], bbc[:, :], op=OP.mult)
                    h_t = sp.tile([P, TBLK], bf16, tag="h")
                    nc.vector.tensor_tensor_scan(h_t[:, :], a_t[:, :], b_t[:, :],
                                                 initial=carry[:, n : n + 1],
                                                 op0=OP.mult, op1=OP.add)
                    nc.vector.tensor_copy(carry[:, n : n + 1], h_t[:, TBLK - 1 : TBLK])
                    cbc = sp.tile([P, TBLK], bf16, tag="cbc")
                    nc.sync.dma_start(cbc[:, :], bcast_row(dbc_out[hh], RK + DS + n, hl, hl + TBLK))
                    yc = sp.tile([P, TBLK], bf16, tag="a")
                    nc.vector.tensor_tensor(yc[:, :], h_t[:, :], cbc[:, :], op=OP.mult)
                    if ys is None:
                        ys = yc
                    else:
                        ys2 = sp.tile([P, TBLK], bf16, tag="ysv")
                        nc.vector.tensor_tensor(ys2[:, :], ys[:, :], yc[:, :], op=OP.add)
                        ys = ys2
                # epilogue: y = (ys + ua*D_skip) * silu(z)
                skip = sp.tile([P, TBLK], bf16, tag="b")
                nc.vector.tensor_scalar(skip[:, :], ua_sb[:, lo : lo + TBLK],
                                        smallp_sb[:, 2:3], None, op0=OP.mult)
                tot = sp.tile([P, TBLK], bf16, tag="h")
                nc.vector.tensor_tensor(tot[:, :], ys[:, :], skip[:, :], op=OP.add)
                zg_t = sp.tile([P, TBLK], bf16, tag="cbc")
                nc.sync.dma_start(zg_t[:, :], zg_dram[:, lo : lo + TBLK])
                y_t = sp.tile([P, TBLK], bf16, tag="y")
                nc.vector.tensor_tensor(y_t[:, :], tot[:, :], zg_t[:, :], op=OP.mult)
                if debug:
                    nc.sync.dma_start(dbg_y[:, lo : lo + TBLK], y_t[:, :])
                # out_proj partials for this block
                for st in range(TBLK // 128):
                    t_abs = lo + st * 128
                    q, r = t_abs // SLAB, t_abs % SLAB
                    po = psB.tile([128, DM], f32, tag="pz")
                    nc.tensor.matmul(po[:, :], lhsT=y_t[:, st * 128 : (st + 1) * 128],
                                     rhs=wout_sb[:, :], start=True, stop=True)
                    ob = wp.tile([128, DM], bf16, tag="ob")
                    nc.scalar.activation(ob[:, :], po[:, :], AF.Copy)
                    nc.sync.dma_start(op_in[q][r : r + 128, :], ob[:, :])
                # issue this slab's ReduceScatter as soon as it completes
                if (blk + 1) * TBLK % SLAB == 0:
                    q = (blk + 1) * TBLK // SLAB - 1
                    nc.gpsimd.collective_compute(
                        "ReduceScatter", OP.add, replica_groups=rg,
                        ins=[op_in[q].ap().opt()], outs=[op_out[q].ap().opt()])

            if debug:
                nc.sync.dma_start(dbg_ua[:, :], ua_sb[:, :])
                nc.sync.dma_start(dbg_dt[:, 0:SH], dt0_sb[:, :])
                nc.sync.dma_start(dbg_dt[:, SH:S], dt1_sb[:, :])
                nc.sync.dma_start(dbg_dbc[:, 0:SH], dbc_out[0][:, :])
                nc.sync.dma_start(dbg_dbc[:, SH:S], dbc_out[1][:, :])
                nc.sync.dma_start(dbg_zg[:, :], zg_dram[:, :])

            # ---- phase 4: LayerNorm + residual per RS chunk ----
            for q in range(NRS):
                emit_ln(q)

    # All ACT functions used (Exp, Ln, Copy, Square, Identity) live in the
    # single "natural_log_exp_and_others" table; restricting the table list
    # stops the load-insertion pass from thrashing between tables.
    import concourse.bacc as bacc_mod
    orig_tables = bacc_mod.get_activation_tables

    def _one_table(arch):
        # keep positions (act_func_set_id is positional) but make all other
        # tables unusable so the pass can't thrash between them
        t = orig_tables(arch)
        return {k: (v if k == "natural_log_exp_and_others" else set()) for k, v in t.items()}

    bacc_mod.get_activation_tables = _one_table
    try:
        nc.compile()
    finally:
        bacc_mod.get_activation_tables = orig_tables
    return nc


def _get_nc():
    if "nc" not in _CACHE:
        _CACHE["nc"] = _build()
    return _CACHE["nc"]


def _make_in_maps(inputs):
    x = np.ascontiguousarray(np.asarray(inputs["x"], dtype=np.float32))
    seq_idx = np.asarray(inputs["seq_idx"], dtype=np.int64)
    W_in = np.asarray(inputs["W_in"], dtype=np.float32)
    conv_w = np.asarray(inputs["conv_w"], dtype=np.float32)
    conv_b = np.asarray(inputs["conv_b"], dtype=np.float32)
    W_xproj = np.asarray(inputs["W_xproj"], dtype=np.float32)
    W_dt = np.asarray(inputs["W_dt"], dtype=np.float32)
    b_dt = np.asarray(inputs["b_dt"], dtype=np.float32)
    A_log = np.asarray(inputs["A_log"], dtype=np.float32)
    D_skip = np.asarray(inputs["D_skip"], dtype=np.float32)
    W_out = np.asarray(inputs["W_out"], dtype=np.float32)
    ln_w = np.asarray(inputs["ln_w"], dtype=np.float32)
    ln_b = np.asarray(inputs["ln_b"], dtype=np.float32)

    # ln scale/bias are identity by construction; they are baked into the graph.
    assert np.allclose(ln_w, 1.0) and np.allclose(ln_b, 0.0), "non-identity LN params unsupported"

    x_chk = x[seq_idx]                              # [S, DM]
    xT = np.ascontiguousarray(x_chk.T).astype(BF16)  # [DM, S]

    in_maps = []
    for i in range(NCORE):
        cs = slice(i * P, (i + 1) * P)
        wuz = np.concatenate([W_in[:, cs], W_in[:, DI + i * P : DI + (i + 1) * P]], axis=1)
        in_maps.append({
            "xT": xT,
            "wuz": np.ascontiguousarray(wuz).astype(BF16),
            "wxp": np.ascontiguousarray(W_xproj[cs]).astype(BF16),
            "wdt": np.ascontiguousarray(W_dt[:, cs]).astype(BF16),
            "wout": np.ascontiguousarray(W_out[cs]).astype(BF16),
            "convw": np.ascontiguousarray(conv_w[cs]),
            "smallp": np.ascontiguousarray(
                np.stack([conv_b[cs], b_dt[cs], D_skip[cs]], axis=1).astype(np.float32)),
            "alog": np.ascontiguousarray(A_log[cs]),
            "xres": np.ascontiguousarray(x_chk[_core_rows(i)]),
        })
    return x, seq_idx, in_maps


def _core_rows(i):
    """Absolute check-node indices held by core i's output, in output order."""
    SLAB = S // NRS
    SHARE = SLAB // NCORE
    return np.concatenate(
        [np.arange(q * SLAB + i * SHARE, q * SLAB + (i + 1) * SHARE) for q in range(NRS)])


def kernel(**inputs):
    from concourse.bass_utils import run_bass_kernel_spmd

    x, seq_idx, in_maps = _make_in_maps(inputs)
    nc = _get_nc()
    trace = bool(int(os.environ.get("KERNEL_TRACE", "0")))
    res = run_bass_kernel_spmd(nc, in_maps, core_ids=list(range(NCORE)), trace=trace)
    if trace:
        _CACHE["last_exec_time_ns"] = res.exec_time_ns
        _CACHE["last_results"] = res
    y = np.empty((S, DM), np.float32)
    for i in range(NCORE):
        y[_core_rows(i)] = np.asarray(res.results[i]["out"])
    outp = x.copy()
    outp[seq_idx] = y
    return outp

